# revision 30
# baseline (speedup 1.0000x reference)
"""Trainium2 Bass kernel for nn_BinaryPathEncoder.

Math: out[n] = prod_k W_{b_k(pos_n)}^T (product over the binary digits of
pos_n below its leading 1; W_0/W_1 = expm(herm_b), pad -> identity).

Let G_b = W_b^T = expm(-herm_b), M(h) = G_{b_0(h)} @ G_{b_1(h)} @ ...
Split pos = hi*256 + lo:
  hi >= 1:  out = A8(lo) @ M(hi)   (8 low bits all valid)
  hi == 0:  out = I @ M(pos)
Tables (per core, identical SPMD program):
  - G via scaling-squaring Taylor in fp32; G/GT stored fp32r
  - A2/A4/A4T doubling tables, M(1..15) chains: fp32r matmuls
  - stat[lo] = A8(lo)^T = A4T[lo>>4] @ A4T[lo&15] -> bf16 DRAM table
  - btab[h] = M(h): M(16q+m) = A4(m) @ M(q) -> bf16 SBUF [P, 16(q), 16(m), P]
Position loop, superblocks of 16 slots (1 block16 or 4 block4s):
  - stationary staged by dyn-offset DMA from the DRAM stat table (sync)
  - moving operands gathered from SBUF btab by dyn-offset engine copies
    split across vector/scalar/gpsimd (reg_load of index batches per engine)
  - 4 static matmuls [128,512] bf16 -> 2 PSUM [128,1024] tiles
  - evac fp32->bf16 split vector/scalar; bf16 out DMA (gpsimd issue)
Host converts bf16->fp32 and scatters slots back to input order.
"""

import contextlib
import os

import numpy as np

import concourse.bass as bass
import concourse.bacc as bacc
import concourse.mybir as mybir
import concourse.tile as tile
import concourse.tile_utils as tile_utils
tile_utils.max_sbuf_usage = 206 * 1024
from concourse.bass_utils import run_bass_kernel_spmd
from concourse.masks import make_identity

FP = mybir.dt.float32
FR = mybir.dt.float32r
BF = mybir.dt.bfloat16
I32 = mybir.dt.int32
P = 128
NCORES = 8
S_EXP = 5          # scaling-squaring: X = -H / 2^S_EXP
ORDER = 12         # Taylor order (||H||~37 -> tail ~1e-8)
NB = 256           # table entries
IDENT_ENTRY = 256  # stationary-table entry holding the identity

# slots gathered by one hardware dma_gather per superblock (from DRAM btab);
# 0 = disabled (the InstDMAGatherAnt path crashes under 8-core SPMD here)
NDMA = int(os.environ.get("NDMA", "0"))
# remaining slots gathered by engine copies (vector, scalar, gpsimd)
GSPLIT = tuple(int(x) for x in os.environ.get("GSPLIT", "6,4,6").split(","))
assert NDMA + sum(GSPLIT) == 16
# evac split: first EVAC_DVE columns (of 2048) on vector, rest on scalar
EVAC_DVE = int(os.environ.get("EVAC_DVE", "1024"))

_prog_cache = {}
_last_ctx = None


def _mm(nc, out, lhsT, rhs):
    nc.tensor.matmul(out, lhsT=lhsT, rhs=rhs, start=True, stop=True)


def _build_expm(nc, consts, psB, scratch, praw, ident):
    """Return (G, GT) fp32r tile pairs: G_b = expm(-H_b), GT_b = G_b^T.

    Interleaves the b=0/b=1 chains to hide serial latency. Taylor recurrence
    T <- I + (X/k) @ T with pre-scaled copies of X^T, identity added on DVE.
    """
    xtj, t, u = {}, {}, {}
    for b in range(2):
        pb = praw[:, b, :]
        ps_t = psB.tile([P, 512], FP, tag="psb")
        nc.tensor.transpose(out=ps_t[:, :P], in_=pb, identity=ident[:])
        xt = scratch.tile([P, P], FP, tag=f"xt{b}")
        nc.vector.tensor_tensor(
            out=xt[:], in0=pb, in1=ps_t[:, :P], op=mybir.AluOpType.subtract
        )
        xj = consts.tile([P, ORDER, P], FP, tag=f"xtj{b}")
        for k in range(1, ORDER + 1):
            s = 1.0 / ((1 << S_EXP) * k)
            if k % 2 == 0:
                nc.vector.tensor_scalar_mul(xj[:, k - 1, :], xt[:], s)
            else:
                nc.scalar.mul(xj[:, k - 1, :], xt[:], s)
        xtj[b] = xj
        tc0 = scratch.tile([P, P], FP, tag=f"tay{b}")
        nc.vector.tensor_copy(tc0[:], ident[:])
        t[b] = tc0
    for k in range(ORDER, 0, -1):
        for b in range(2):
            ps = psB.tile([P, 512], FP, tag="psb")
            _mm(nc, ps[:, :P], xtj[b][:, k - 1, :], t[b][:])
            tn = scratch.tile([P, P], FP, tag=f"tay{b}")
            nc.vector.tensor_tensor(
                out=tn[:], in0=ps[:, :P], in1=ident[:], op=mybir.AluOpType.add
            )
            t[b] = tn
    for b in range(2):
        ps_u = psB.tile([P, 512], FP, tag="psb")
        nc.tensor.transpose(out=ps_u[:, :P], in_=t[b][:], identity=ident[:])
        uc = scratch.tile([P, P], FP, tag=f"tayu{b}")
        nc.scalar.copy(uc[:], ps_u[:, :P])
        u[b] = uc
    for _ in range(S_EXP):
        for b in range(2):
            ps1 = psB.tile([P, 512], FP, tag="psb")
            ps2 = psB.tile([P, 512], FP, tag="psb")
            _mm(nc, ps1[:, :P], u[b][:], t[b][:])   # T' = T @ T
            _mm(nc, ps2[:, :P], t[b][:], u[b][:])   # U' = (T@T)^T
            tn = scratch.tile([P, P], FP, tag=f"tay{b}")
            un = scratch.tile([P, P], FP, tag=f"tayu{b}")
            nc.vector.tensor_copy(tn[:], ps1[:, :P])
            nc.scalar.copy(un[:], ps2[:, :P])
            t[b], u[b] = tn, un
    Gs, GTs = [], []
    for b in range(2):
        g = consts.tile([P, P], FR, tag=f"g{b}")
        gt = consts.tile([P, P], FR, tag=f"gt{b}")
        nc.vector.tensor_copy(g[:], t[b][:])
        nc.scalar.copy(gt[:], u[b][:])
        Gs.append(g)
        GTs.append(gt)
    return Gs, GTs


def build_program(n16, n4):
    assert n4 % 4 == 0
    nblk = n16 + n4
    nslots = n16 * 16 + n4 * 4
    nsb = n16 + n4 // 4          # superblocks of 16 slots
    nc = bacc.Bacc("TRN2", target_bir_lowering=False, debug=False,
                   num_devices=NCORES)
    ncols16 = NDMA * P // 16     # int16 index columns per superblock
    praw_d = nc.dram_tensor("praw", [2, P, P], FP, kind="ExternalInput")
    sioff_d = nc.dram_tensor("sioff", [1, nblk], I32, kind="ExternalInput")
    bidx_d = nc.dram_tensor("bidx", [1, nslots], I32, kind="ExternalInput")
    out_d = nc.dram_tensor("out", [P, nslots * P], BF, kind="ExternalOutput")
    stat_d = nc.dram_tensor("stat", [(NB + 1) * P, P], BF)
    if NDMA > 0:
        bidx16_d = nc.dram_tensor("bidx16", [P, nsb * ncols16],
                                  mybir.dt.int16, kind="ExternalInput")
        # btab DRAM copy, row (p*256 + e) = row p of entry e -> the
        # SBUF->DRAM spill is 128 contiguous 64KB runs; gather idx fits int16
        btd_d = nc.dram_tensor("btd", [P * NB, P], BF)

    with tile.TileContext(nc) as tc:
        with (
            tc.tile_pool(name="consts", bufs=1) as consts,
            tc.tile_pool(name="scratch", bufs=2) as scratch,
            tc.tile_pool(name="atab", bufs=1) as atab,
            tc.tile_pool(name="btab", bufs=1) as btabp,
            tc.tile_pool(name="sstage", bufs=4) as sstagep,
            tc.tile_pool(name="stage", bufs=int(os.environ.get("STAGE_BUFS", "8"))) as stagep,
            tc.tile_pool(name="mv", bufs=int(os.environ.get("MV_BUFS", "3"))) as mvp,
            tc.tile_pool(name="obuf", bufs=int(os.environ.get("OBUF_BUFS", "3"))) as obufp,
            tc.tile_pool(name="psB", bufs=int(os.environ.get("PSB_BUFS", "4")), space="PSUM") as psB,
            tc.tile_pool(name="psP", bufs=int(os.environ.get("PSP_BUFS", "2")), space="PSUM") as psP,
        ):
            ident = consts.tile([P, P], FP, tag="ident")
            make_identity(nc, ident[:])
            praw = consts.tile([P, 2, P], FP, tag="praw")
            nc.sync.dma_start(praw[:], praw_d[:].rearrange("p r c -> r p c"))
            sioff = consts.tile([1, nblk], I32, tag="sioff")
            bidx = consts.tile([1, nslots], I32, tag="bidx")
            nc.sync.dma_start(sioff[:], sioff_d[:])
            nc.sync.dma_start(bidx[:], bidx_d[:])
            if NDMA > 0:
                bidx16 = consts.tile([P, nsb * ncols16], mybir.dt.int16,
                                     tag="bidx16")
                nc.sync.dma_start(bidx16[:], bidx16_d[:])

            # ---- phase A: primitives ----
            G, GT = _build_expm(nc, consts, psB, scratch, praw, ident)

            # ---- phases A2+B interleaved: M(1..15) chains and A2/A4 tables
            # (fp32r).  The m15 levels are serial (mm -> evac -> mm); the a2/a4
            # mms are emitted between the levels so the PE never stalls on the
            # m15 evacs (tile executes per-engine in emission order).
            m15 = atab.tile([P, 16, P], FR, tag="m15")
            btab = btabp.tile([P, 16, 16, P], BF, tag="btab")
            nc.vector.tensor_copy(m15[:, 1, :], ident[:])
            nc.vector.tensor_copy(m15[:, 2, :], G[0][:])
            nc.vector.tensor_copy(m15[:, 3, :], G[1][:])
            nc.scalar.copy(btab[:, 0, 0, :], ident[:])
            nc.scalar.copy(btab[:, 0, 1, :], ident[:])
            nc.scalar.copy(btab[:, 0, 2, :], G[0][:])
            nc.scalar.copy(btab[:, 0, 3, :], G[1][:])
            m15v = m15[:].rearrange("r (c b2) p -> r c b2 p", b2=2)
            btsv = btab[:, 0, :, :].rearrange("r (c b2) p -> r c b2 p", b2=2)

            def m15_level(c0, c1):
                # entries 2c+b for c in [c0, c1): M(2c+b) = G_b @ M(c)
                ncols = c1 - c0
                for b in range(2):
                    ps = psB.tile([P, 512], FP, tag="psb")
                    _mm(nc, ps[:, : ncols * P], GT[b][:],
                        m15[:, c0:c1, :].rearrange("r m c -> r (m c)"))
                    psv = ps[:, : ncols * P].rearrange("r (m c) -> r m c", c=P)
                    nc.vector.tensor_copy(m15v[:, c0:c1, b, :], psv)
                    nc.scalar.copy(btsv[:, c0:c1, b, :], psv)

            a2 = atab.tile([P, 4, P], FR, tag="a2")
            a2t = atab.tile([P, 4, P], FR, tag="a2t")
            m15_level(2, 4)
            for m in range(4):
                ps = psB.tile([P, 512], FP, tag="psb")
                _mm(nc, ps[:, :P], GT[m & 1][:], G[m >> 1][:])   # A2[m]
                nc.vector.tensor_copy(a2[:, m, :], ps[:, :P])
                ps2 = psB.tile([P, 512], FP, tag="psb")
                _mm(nc, ps2[:, :P], G[m >> 1][:], GT[m & 1][:])  # A2T[m]
                nc.scalar.copy(a2t[:, m, :], ps2[:, :P])
                if m == 1:
                    m15_level(4, 8)
            a4 = atab.tile([P, 16, P], FR, tag="a4")
            a4t = atab.tile([P, 16, P], FR, tag="a4t")
            a2f = a2[:].rearrange("r m c -> r (m c)")
            a2tf = a2t[:].rearrange("r m c -> r (m c)")
            for a in range(4):
                ps = psB.tile([P, 512], FP, tag="psb")
                _mm(nc, ps[:], a2t[:, a, :], a2f)        # A4[a+4b] over b
                for b2 in range(4):
                    nc.vector.tensor_copy(
                        a4[:, a + 4 * b2, :], ps[:, b2 * P : (b2 + 1) * P]
                    )
                # A4T[m] = A2T[m>>2] @ A2T[m&3]; fix a=m>>2: m = 4a+b contiguous
                ps2 = psB.tile([P, 512], FP, tag="psb")
                _mm(nc, ps2[:], a2[:, a, :], a2tf)
                nc.scalar.copy(
                    a4t[:, 4 * a : 4 * a + 4, :].rearrange("r m c -> r (m c)"),
                    ps2[:],
                )

            # ---- phase C: stat table (A8^T) -> stat_d rows (bf16) ----
            a4tf = a4t[:].rearrange("r m c -> r (m c)")
            stat_v = stat_d[:].rearrange("(e r) c -> r e c", r=P)
            for g in range(16):
                for q in range(4):
                    sst = sstagep.tile([P, 4, P], BF, tag="sst")
                    ps = psB.tile([P, 512], FP, tag="psb")
                    # stat[16g + (4q+j)] = A4T[g] @ A4T[4q+j], j=0..3
                    _mm(nc, ps[:], a4[:, g, :], a4tf[:, q * 512 : (q + 1) * 512])
                    psv = ps[:].rearrange("r (m c) -> r m c", c=P)
                    if (g + q) % 2 == 0:
                        nc.vector.tensor_copy(sst[:], psv)
                    else:
                        nc.scalar.copy(sst[:], psv)
                    nc.sync.dma_start(
                        stat_v[:, 16 * g + 4 * q : 16 * g + 4 * q + 4, :], sst[:]
                    )
            sstI = sstagep.tile([P, 4, P], BF, tag="sst")
            nc.vector.tensor_copy(sstI[:, 0, :], ident[:])
            nc.sync.dma_start(stat_v[:, NB : NB + 1, :], sstI[:, 0:1, :])

            # ---- phase D: btab entries 16..255 = A4(m) @ M(q), fp32r ----
            for m in range(16):
                for (q0, q1) in ((1, 5), (5, 9), (9, 13), (13, 16)):
                    nq = q1 - q0
                    ps = psB.tile([P, 512], FP, tag="psb")
                    _mm(nc, ps[:, : nq * P], a4t[:, m, :],
                        m15[:, q0:q1, :].rearrange("r m c -> r (m c)"))
                    psv = ps[:, : nq * P].rearrange("r (m c) -> r m c", c=P)
                    if (m + q0) % 2 == 0:
                        nc.vector.tensor_copy(btab[:, q0:q1, m, :], psv)
                    else:
                        nc.scalar.copy(btab[:, q0:q1, m, :], psv)

            if NDMA > 0:
                # spill btab to DRAM rows (p*256+e): 64KB runs per partition
                nc.sync.dma_start(
                    btd_d[:].rearrange("(r e) c -> r (e c)", r=P), btab[:]
                )

            # ---- phase E: position loop ----
            btf = btab[:].rearrange("r q m p -> r (q m p)")
            nV, nA, nG = GSPLIT
            with contextlib.ExitStack() as regctx:
                vregs = [regctx.enter_context(nc.vector.register(f"rv{j}"))
                         for j in range(nV)]
                aregs = [regctx.enter_context(nc.scalar.register(f"ra{j}"))
                         for j in range(nA)]
                gregs = [regctx.enter_context(nc.gpsimd.register(f"rg{j}"))
                         for j in range(nG)]
                rs = regctx.enter_context(nc.sync.register("rs"))
                if NDMA > 0:
                    rnum = regctx.enter_context(nc.gpsimd.register("rnum"))
                    nc.gpsimd.reg_mov(rnum, NDMA * P)
                    vnum = nc.gpsimd.snap(rnum, donate=True,
                                          min_val=NDMA * P, max_val=NDMA * P)

                def stage_st(blk):
                    st = stagep.tile([P, P], BF, tag="st")
                    nc.sync.reg_load(rs, sioff[0:1, blk : blk + 1])
                    so = nc.sync.snap(rs, min_val=0, max_val=NB * P)
                    nc.sync.dma_start(st[:], stat_d[bass.ds(so, P), :])
                    return st

                def gather(eng, regs, mv, s0, j0, cnt):
                    if cnt == 0:
                        return
                    eng.reg_load(regs[:cnt], bidx[0:1, s0 + j0 : s0 + j0 + cnt])
                    for i in range(cnt):
                        off = eng.snap(regs[i], donate=True,
                                       min_val=0, max_val=(NB - 1) * P)
                        src = btf[:, bass.ds(off, P)]
                        if eng is nc.scalar:
                            eng.copy(mv[:, j0 + i, :], src)
                        else:
                            eng.tensor_copy(mv[:, j0 + i, :], src)

                for sb in range(nsb):
                    s0 = sb * 16
                    if sb < n16:
                        sts = [stage_st(sb)] * 4
                    else:
                        t4 = sb - n16
                        sts = [stage_st(n16 + 4 * t4 + g) for g in range(4)]
                    mv = mvp.tile([P, 16, P], BF, tag="mv")
                    if NDMA > 0:
                        nc.gpsimd.dma_gather(
                            out_ap=mv[:, :NDMA, :],
                            in_ap=btd_d[:],
                            idxs_ap=bidx16[:, sb * ncols16 : (sb + 1) * ncols16],
                            num_idxs=NDMA * P,
                            num_idxs_reg=vnum,
                            elem_size=P,
                        )
                    gather(nc.vector, vregs, mv, s0, NDMA, nV)
                    gather(nc.scalar, aregs, mv, s0, NDMA + nV, nA)
                    gather(nc.gpsimd, gregs, mv, s0, NDMA + nV + nA, nG)
                    pts = [psP.tile([P, 1024], FP, tag="pp", name=f"pp{h}")
                           for h in range(2)]
                    for q in range(4):
                        _mm(nc, pts[q // 2][:, (q % 2) * 512 : (q % 2 + 1) * 512],
                            sts[q][:],
                            mv[:, 4 * q : 4 * q + 4, :].rearrange("r m c -> r (m c)"))
                    ob = obufp.tile([P, 16 * P], BF, tag="ob")
                    # vector evacs ob[:, :EVAC_DVE], scalar the rest
                    lo = min(EVAC_DVE, 1024)
                    if lo > 0:
                        nc.vector.tensor_copy(ob[:, :lo], pts[0][:, :lo])
                    if lo < 1024:
                        nc.scalar.copy(ob[:, lo:1024], pts[0][:, lo:])
                    hi = max(EVAC_DVE, 1024)
                    if hi > 1024:
                        nc.vector.tensor_copy(
                            ob[:, 1024:hi], pts[1][:, : hi - 1024])
                    if hi < 2048:
                        nc.scalar.copy(ob[:, hi:], pts[1][:, hi - 1024 :])
                    nc.gpsimd.dma_start(
                        out_d[:, s0 * P : (s0 + 16) * P], ob[:]
                    )
    nc.compile()
    return nc


def _plan_blocks(unique):
    """Pack positions into 16-blocks and 4-blocks sharing a stationary entry."""
    lo = unique & 255
    hi = unique >> 8
    ent = np.where(hi > 0, lo, IDENT_ENTRY)
    bent = np.where(hi > 0, hi, unique)  # hi==0 -> out = I @ M(pos)
    order = np.argsort(ent, kind="stable")
    es = ent[order]
    bounds = np.flatnonzero(np.r_[True, es[1:] != es[:-1], True])

    blocks16, blocks4 = [], []
    for s, e in zip(bounds[:-1], bounds[1:]):
        idxs = order[s:e]
        v = int(es[s])
        g = len(idxs)
        q0 = 0
        while g - q0 >= 16:
            blocks16.append((v, idxs[q0 : q0 + 16]))
            q0 += 16
        while q0 < g:
            blocks4.append((v, idxs[q0 : q0 + 4]))
            q0 += 4
    return blocks16, blocks4, bent


def kernel(unique, primitives_raw, identity=None, **_):
    unique = np.asarray(unique)
    praw = np.ascontiguousarray(np.asarray(primitives_raw, np.float32))

    blocks16, blocks4, bent = _plan_blocks(unique.astype(np.int64))
    n16 = -(-len(blocks16) // NCORES)
    n4 = -(-len(blocks4) // (NCORES * 4)) * 4
    while len(blocks16) < NCORES * n16:
        blocks16.append((IDENT_ENTRY, np.empty(0, np.int64)))
    while len(blocks4) < NCORES * n4:
        blocks4.append((IDENT_ENTRY, np.empty(0, np.int64)))
    nslots = n16 * 16 + n4 * 4

    nsb = n16 + n4 // 4
    ncols16 = NDMA * P // 16
    slot_of_pos = np.zeros(unique.shape[0], np.int64)
    sioff = np.zeros((NCORES, n16 + n4), np.int32)
    ent_of_slot = np.zeros((NCORES, nslots), np.int32)  # bent per slot
    for i, (v, mem) in enumerate(blocks16):
        c, k = divmod(i, n16)
        sioff[c, k] = v * P
        for j, pidx in enumerate(mem):
            ent_of_slot[c, k * 16 + j] = int(bent[pidx])
            slot_of_pos[pidx] = c * nslots + k * 16 + j
    for i, (v, mem) in enumerate(blocks4):
        c, k = divmod(i, n4)
        sioff[c, n16 + k] = v * P
        base = n16 * 16 + k * 4
        for j, pidx in enumerate(mem):
            ent_of_slot[c, base + j] = int(bent[pidx])
            slot_of_pos[pidx] = c * nslots + base + j
    # engine-gather offsets (elements into the SBUF btab)
    bidx = (ent_of_slot[:, None, :] * P).astype(np.int32)
    bidx16 = None
    if NDMA > 0:
        # dma_gather int16 indices: slot g of superblock sb -> 128 indices
        # i = g*128+p valued p*256+e, laid out at [i%16, sb*ncols16 + i//16]
        es = ent_of_slot.reshape(NCORES, nsb, 16)[:, :, :NDMA]
        p = np.arange(P)
        idxval = (p[None, None, None, :] * NB + es[..., None]).astype(np.int64)
        i_lin = (np.arange(NDMA)[:, None] * P + p[None, :])      # [NDMA, P]
        bidx16 = np.zeros((NCORES, 16, nsb * ncols16), np.int16)
        rows = (i_lin % 16).ravel()
        cols = (i_lin // 16).ravel()
        for c in range(NCORES):
            for sb in range(nsb):
                vals = idxval[c, sb].ravel()                     # [NDMA*P]
                bidx16[c, rows, sb * ncols16 + cols] = vals
        # each Q7 core pair reads its own partitions: replicate the block 8x
        bidx16 = np.tile(bidx16, (1, 8, 1))                      # [C, 128, ..]

    key = (n16, n4)
    if key not in _prog_cache:
        _prog_cache[key] = build_program(n16, n4)
    nc = _prog_cache[key]

    in_maps = [
        {
            "praw": praw,
            "sioff": np.ascontiguousarray(sioff[c].reshape(1, -1)),
            "bidx": np.ascontiguousarray(bidx[c]),
            **({"bidx16": np.ascontiguousarray(bidx16[c])} if NDMA > 0 else {}),
        }
        for c in range(NCORES)
    ]
    global _last_ctx
    _last_ctx = (nc, in_maps)
    res = run_bass_kernel_spmd(nc, in_maps, list(range(NCORES)))
    outs = np.concatenate(
        [
            np.asarray(res.results[c]["out"])
            .reshape(P, nslots, P)
            .transpose(1, 0, 2)
            for c in range(NCORES)
        ],
        axis=0,
    )
    return np.ascontiguousarray(outs[slot_of_pos]).astype(np.float32)


if __name__ == "__main__":
    rng = np.random.default_rng(0)
    u = rng.integers(1, 65536, 64).astype(np.int32)
    pr = rng.random((2, P, P), np.float32)
    o = kernel(u, pr)
    print(o.shape, o.dtype)


# revision 39
# speedup vs baseline: 1.1281x; 1.1281x over previous
"""Trainium2 Bass kernel for nn_BinaryPathEncoder.

Math: out[n] = prod_k W_{b_k(pos_n)}^T (product over the binary digits of
pos_n below its leading 1; W_0/W_1 = expm(herm_b), pad -> identity).

Let G_b = W_b^T = expm(-herm_b), M(h) = G_{b_0(h)} @ G_{b_1(h)} @ ...
Split pos = hi*256 + lo:
  hi >= 1:  out = A8(lo) @ M(hi)   (8 low bits all valid)
  hi == 0:  out = I @ M(pos)
Tables (per core, identical SPMD program):
  - G via scaling-squaring Taylor in fp32; G/GT stored fp32r
  - A2/A4/A4T doubling tables, M(1..15) chains: fp32r matmuls
  - stat[lo] = A8(lo)^T = A4T[lo>>4] @ A4T[lo&15] -> bf16 DRAM table
  - btab[h] = M(h): M(16q+m) = A4(m) @ M(q) -> bf16 SBUF [P, 16(q), 16(m), P]
Position loop, superblocks of 16 slots (1 block16 or 4 block4s):
  - stationary staged by dyn-offset DMA from the DRAM stat table (sync)
  - moving operands gathered from SBUF btab by dyn-offset engine copies
    split across vector/scalar/gpsimd (reg_load of index batches per engine)
  - 4 static matmuls [128,512] bf16 -> 2 PSUM [128,1024] tiles
  - evac fp32->bf16 split vector/scalar; bf16 out DMA (gpsimd issue)
Host converts bf16->fp32 and scatters slots back to input order.
"""

import contextlib
import os

import numpy as np

import concourse.bass as bass
import concourse.bacc as bacc
import concourse.mybir as mybir
import concourse.tile as tile
import concourse.tile_utils as tile_utils
tile_utils.max_sbuf_usage = 206 * 1024
from concourse.bass_utils import run_bass_kernel_spmd
from concourse.masks import make_identity

FP = mybir.dt.float32
FR = mybir.dt.float32r
BF = mybir.dt.bfloat16
I32 = mybir.dt.int32
P = 128
NCORES = 8
S_EXP = 5          # scaling-squaring: X = -H / 2^S_EXP
ORDER = 12         # Taylor order (||H||~37 -> tail ~1e-8)
NB = 256           # table entries
IDENT_ENTRY = 256  # stationary-table entry holding the identity

# slots gathered by one hardware dma_gather per superblock (from DRAM btab);
# 0 = disabled (the InstDMAGatherAnt path crashes under 8-core SPMD here)
NDMA = int(os.environ.get("NDMA", "0"))
# remaining slots gathered by engine copies (vector, scalar, gpsimd)
GSPLIT = tuple(int(x) for x in os.environ.get("GSPLIT", "6,5,5").split(","))
assert NDMA + sum(GSPLIT) == 16
# evac split: first EVAC_DVE columns (of 2048) on vector, rest on scalar
EVAC_DVE = int(os.environ.get("EVAC_DVE", "1024"))

_prog_cache = {}
_last_ctx = None


def _mm(nc, out, lhsT, rhs):
    nc.tensor.matmul(out, lhsT=lhsT, rhs=rhs, start=True, stop=True)


def _build_expm(nc, consts, psB, scratch, praw, ident):
    """Return (G, GT) fp32r tile pairs: G_b = expm(-H_b), GT_b = G_b^T.

    Interleaves the b=0/b=1 chains to hide serial latency. Taylor recurrence
    T <- I + (X/k) @ T with pre-scaled copies of X^T, identity added on DVE.
    """
    xtj, t, u = {}, {}, {}
    for b in range(2):
        pb = praw[:, b, :]
        ps_t = psB.tile([P, 512], FP, tag="psb")
        nc.tensor.transpose(out=ps_t[:, :P], in_=pb, identity=ident[:])
        xt = scratch.tile([P, P], FP, tag=f"xt{b}")
        nc.vector.tensor_tensor(
            out=xt[:], in0=pb, in1=ps_t[:, :P], op=mybir.AluOpType.subtract
        )
        xj = consts.tile([P, ORDER, P], FP, tag=f"xtj{b}")
        for k in range(1, ORDER + 1):
            s = 1.0 / ((1 << S_EXP) * k)
            if k % 2 == 0:
                nc.vector.tensor_scalar_mul(xj[:, k - 1, :], xt[:], s)
            else:
                nc.scalar.mul(xj[:, k - 1, :], xt[:], s)
        xtj[b] = xj
        tc0 = scratch.tile([P, P], FP, tag=f"tay{b}")
        nc.vector.tensor_copy(tc0[:], ident[:])
        t[b] = tc0
    for k in range(ORDER, 0, -1):
        for b in range(2):
            ps = psB.tile([P, 512], FP, tag="psb")
            _mm(nc, ps[:, :P], xtj[b][:, k - 1, :], t[b][:])
            tn = scratch.tile([P, P], FP, tag=f"tay{b}")
            nc.vector.tensor_tensor(
                out=tn[:], in0=ps[:, :P], in1=ident[:], op=mybir.AluOpType.add
            )
            t[b] = tn
    for b in range(2):
        ps_u = psB.tile([P, 512], FP, tag="psb")
        nc.tensor.transpose(out=ps_u[:, :P], in_=t[b][:], identity=ident[:])
        uc = scratch.tile([P, P], FP, tag=f"tayu{b}")
        nc.scalar.copy(uc[:], ps_u[:, :P])
        u[b] = uc
    for _ in range(S_EXP):
        for b in range(2):
            ps1 = psB.tile([P, 512], FP, tag="psb")
            ps2 = psB.tile([P, 512], FP, tag="psb")
            _mm(nc, ps1[:, :P], u[b][:], t[b][:])   # T' = T @ T
            _mm(nc, ps2[:, :P], t[b][:], u[b][:])   # U' = (T@T)^T
            tn = scratch.tile([P, P], FP, tag=f"tay{b}")
            un = scratch.tile([P, P], FP, tag=f"tayu{b}")
            nc.vector.tensor_copy(tn[:], ps1[:, :P])
            nc.scalar.copy(un[:], ps2[:, :P])
            t[b], u[b] = tn, un
    Gs, GTs = [], []
    for b in range(2):
        g = consts.tile([P, P], FR, tag=f"g{b}")
        gt = consts.tile([P, P], FR, tag=f"gt{b}")
        nc.vector.tensor_copy(g[:], t[b][:])
        nc.scalar.copy(gt[:], u[b][:])
        Gs.append(g)
        GTs.append(gt)
    return Gs, GTs


def build_program(n16, n4):
    assert n4 % 4 == 0
    nblk = n16 + n4
    nslots = n16 * 16 + n4 * 4
    nsb = n16 + n4 // 4          # superblocks of 16 slots
    nc = bacc.Bacc("TRN2", target_bir_lowering=False, debug=False,
                   num_devices=NCORES)
    ncols16 = NDMA * P // 16     # int16 index columns per superblock
    praw_d = nc.dram_tensor("praw", [2, P, P], FP, kind="ExternalInput")
    sioff_d = nc.dram_tensor("sioff", [1, nblk], I32, kind="ExternalInput")
    # row 0: bf16-element offsets (scalar copies); row 1: int32-element
    # offsets (vector/gpsimd copies run bitcast-to-int32, halving elems)
    bidx_d = nc.dram_tensor("bidx", [2, nslots], I32, kind="ExternalInput")
    out_d = nc.dram_tensor("out", [P, nslots * P], BF, kind="ExternalOutput")
    stat_d = nc.dram_tensor("stat", [(NB + 1) * P, P], BF)
    if NDMA > 0:
        bidx16_d = nc.dram_tensor("bidx16", [P, nsb * ncols16],
                                  mybir.dt.int16, kind="ExternalInput")
        # btab DRAM copy, row (p*256 + e) = row p of entry e -> the
        # SBUF->DRAM spill is 128 contiguous 64KB runs; gather idx fits int16
        btd_d = nc.dram_tensor("btd", [P * NB, P], BF)

    with tile.TileContext(nc) as tc:
        with (
            tc.tile_pool(name="consts", bufs=1) as consts,
            tc.tile_pool(name="scratch", bufs=2) as scratch,
            tc.tile_pool(name="atab", bufs=1) as atab,
            tc.tile_pool(name="btab", bufs=1) as btabp,
            tc.tile_pool(name="sstage", bufs=4) as sstagep,
            tc.tile_pool(name="stage", bufs=int(os.environ.get("STAGE_BUFS", "8"))) as stagep,
            tc.tile_pool(name="mv", bufs=int(os.environ.get("MV_BUFS", "4"))) as mvp,
            tc.tile_pool(name="obuf", bufs=int(os.environ.get("OBUF_BUFS", "4"))) as obufp,
            tc.tile_pool(name="psB", bufs=int(os.environ.get("PSB_BUFS", "4")), space="PSUM") as psB,
            tc.tile_pool(name="psP", bufs=int(os.environ.get("PSP_BUFS", "2")), space="PSUM") as psP,
        ):
            ident = consts.tile([P, P], FP, tag="ident")
            make_identity(nc, ident[:])
            praw = consts.tile([P, 2, P], FP, tag="praw")
            nc.sync.dma_start(praw[:], praw_d[:].rearrange("p r c -> r p c"))
            sioff = consts.tile([1, nblk], I32, tag="sioff")
            bidx = consts.tile([2, nslots], I32, tag="bidx")
            nc.sync.dma_start(sioff[:], sioff_d[:])
            nc.sync.dma_start(bidx[:], bidx_d[:])
            if NDMA > 0:
                bidx16 = consts.tile([P, nsb * ncols16], mybir.dt.int16,
                                     tag="bidx16")
                nc.sync.dma_start(bidx16[:], bidx16_d[:])

            # ---- phase A: primitives ----
            G, GT = _build_expm(nc, consts, psB, scratch, praw, ident)

            # ---- phases A2+B interleaved: M(1..15) chains and A2/A4 tables
            # (fp32r).  The m15 levels are serial (mm -> evac -> mm); the a2/a4
            # mms are emitted between the levels so the PE never stalls on the
            # m15 evacs (tile executes per-engine in emission order).
            m15 = atab.tile([P, 16, P], FR, tag="m15")
            btab = btabp.tile([P, 16, 16, P], BF, tag="btab")
            nc.vector.tensor_copy(m15[:, 1, :], ident[:])
            nc.vector.tensor_copy(m15[:, 2, :], G[0][:])
            nc.vector.tensor_copy(m15[:, 3, :], G[1][:])
            nc.scalar.copy(btab[:, 0, 0, :], ident[:])
            nc.scalar.copy(btab[:, 0, 1, :], ident[:])
            nc.scalar.copy(btab[:, 0, 2, :], G[0][:])
            nc.scalar.copy(btab[:, 0, 3, :], G[1][:])
            m15v = m15[:].rearrange("r (c b2) p -> r c b2 p", b2=2)
            btsv = btab[:, 0, :, :].rearrange("r (c b2) p -> r c b2 p", b2=2)

            def m15_level(c0, c1):
                # entries 2c+b for c in [c0, c1): M(2c+b) = G_b @ M(c)
                ncols = c1 - c0
                for b in range(2):
                    ps = psB.tile([P, 512], FP, tag="psb")
                    _mm(nc, ps[:, : ncols * P], GT[b][:],
                        m15[:, c0:c1, :].rearrange("r m c -> r (m c)"))
                    psv = ps[:, : ncols * P].rearrange("r (m c) -> r m c", c=P)
                    nc.vector.tensor_copy(m15v[:, c0:c1, b, :], psv)
                    nc.scalar.copy(btsv[:, c0:c1, b, :], psv)

            a2 = atab.tile([P, 4, P], FR, tag="a2")
            a2t = atab.tile([P, 4, P], FR, tag="a2t")
            m15_level(2, 4)
            for m in range(4):
                ps = psB.tile([P, 512], FP, tag="psb")
                _mm(nc, ps[:, :P], GT[m & 1][:], G[m >> 1][:])   # A2[m]
                nc.vector.tensor_copy(a2[:, m, :], ps[:, :P])
                ps2 = psB.tile([P, 512], FP, tag="psb")
                _mm(nc, ps2[:, :P], G[m >> 1][:], GT[m & 1][:])  # A2T[m]
                nc.scalar.copy(a2t[:, m, :], ps2[:, :P])
                if m == 1:
                    m15_level(4, 8)
            a4 = atab.tile([P, 16, P], FR, tag="a4")
            a4t = atab.tile([P, 16, P], FR, tag="a4t")
            a2f = a2[:].rearrange("r m c -> r (m c)")
            a2tf = a2t[:].rearrange("r m c -> r (m c)")
            for a in range(4):
                ps = psB.tile([P, 512], FP, tag="psb")
                _mm(nc, ps[:], a2t[:, a, :], a2f)        # A4[a+4b] over b
                for b2 in range(4):
                    nc.vector.tensor_copy(
                        a4[:, a + 4 * b2, :], ps[:, b2 * P : (b2 + 1) * P]
                    )
                # A4T[m] = A2T[m>>2] @ A2T[m&3]; fix a=m>>2: m = 4a+b contiguous
                ps2 = psB.tile([P, 512], FP, tag="psb")
                _mm(nc, ps2[:], a2[:, a, :], a2tf)
                nc.scalar.copy(
                    a4t[:, 4 * a : 4 * a + 4, :].rearrange("r m c -> r (m c)"),
                    ps2[:],
                )

            # ---- phase C: stat table (A8^T) -> stat_d rows (bf16) ----
            a4tf = a4t[:].rearrange("r m c -> r (m c)")
            stat_v = stat_d[:].rearrange("(e r) c -> r e c", r=P)
            for g in range(16):
                for q in range(4):
                    sst = sstagep.tile([P, 4, P], BF, tag="sst")
                    ps = psB.tile([P, 512], FP, tag="psb")
                    # stat[16g + (4q+j)] = A4T[g] @ A4T[4q+j], j=0..3
                    _mm(nc, ps[:], a4[:, g, :], a4tf[:, q * 512 : (q + 1) * 512])
                    psv = ps[:].rearrange("r (m c) -> r m c", c=P)
                    if (g + q) % 2 == 0:
                        nc.vector.tensor_copy(sst[:], psv)
                    else:
                        nc.scalar.copy(sst[:], psv)
                    nc.sync.dma_start(
                        stat_v[:, 16 * g + 4 * q : 16 * g + 4 * q + 4, :], sst[:]
                    )
            sstI = sstagep.tile([P, 4, P], BF, tag="sst")
            nc.vector.tensor_copy(sstI[:, 0, :], ident[:])
            nc.sync.dma_start(stat_v[:, NB : NB + 1, :], sstI[:, 0:1, :])

            # ---- phase D: btab entries 16..255 = A4(m) @ M(q), fp32r ----
            for m in range(16):
                for (q0, q1) in ((1, 5), (5, 9), (9, 13), (13, 16)):
                    nq = q1 - q0
                    ps = psB.tile([P, 512], FP, tag="psb")
                    _mm(nc, ps[:, : nq * P], a4t[:, m, :],
                        m15[:, q0:q1, :].rearrange("r m c -> r (m c)"))
                    psv = ps[:, : nq * P].rearrange("r (m c) -> r m c", c=P)
                    if (m + q0) % 2 == 0:
                        nc.vector.tensor_copy(btab[:, q0:q1, m, :], psv)
                    else:
                        nc.scalar.copy(btab[:, q0:q1, m, :], psv)

            if NDMA > 0:
                # spill btab to DRAM rows (p*256+e): 64KB runs per partition
                nc.sync.dma_start(
                    btd_d[:].rearrange("(r e) c -> r (e c)", r=P), btab[:]
                )

            # ---- phase E: position loop ----
            btf = btab[:].rearrange("r q m p -> r (q m p)")
            nV, nA, nG = GSPLIT
            with contextlib.ExitStack() as regctx:
                vregs = [regctx.enter_context(nc.vector.register(f"rv{j}"))
                         for j in range(nV)]
                aregs = [regctx.enter_context(nc.scalar.register(f"ra{j}"))
                         for j in range(nA)]
                gregs = [regctx.enter_context(nc.gpsimd.register(f"rg{j}"))
                         for j in range(nG)]
                rs = regctx.enter_context(nc.sync.register("rs"))
                if NDMA > 0:
                    rnum = regctx.enter_context(nc.gpsimd.register("rnum"))
                    nc.gpsimd.reg_mov(rnum, NDMA * P)
                    vnum = nc.gpsimd.snap(rnum, donate=True,
                                          min_val=NDMA * P, max_val=NDMA * P)

                def stage_st(blk):
                    st = stagep.tile([P, P], BF, tag="st")
                    nc.sync.reg_load(rs, sioff[0:1, blk : blk + 1])
                    so = nc.sync.snap(rs, min_val=0, max_val=NB * P)
                    nc.sync.dma_start(st[:], stat_d[bass.ds(so, P), :])
                    return st

                btf32 = btf.bitcast(I32)        # [P, 16384] int32 view

                def gather(eng, regs, mv, s0, j0, cnt):
                    if cnt == 0:
                        return
                    row = 0 if eng is nc.scalar else 1
                    eng.reg_load(regs[:cnt],
                                 bidx[row : row + 1, s0 + j0 : s0 + j0 + cnt])
                    for i in range(cnt):
                        if eng is nc.scalar:
                            off = eng.snap(regs[i], donate=True,
                                           min_val=0, max_val=(NB - 1) * P)
                            eng.copy(mv[:, j0 + i, :], btf[:, bass.ds(off, P)])
                        else:
                            off = eng.snap(regs[i], donate=True,
                                           min_val=0, max_val=(NB - 1) * P // 2)
                            eng.tensor_copy(
                                mv[:, j0 + i, :].bitcast(I32),
                                btf32[:, bass.ds(off, P // 2)],
                            )

                for sb in range(nsb):
                    s0 = sb * 16
                    if sb < n16:
                        sts = [stage_st(sb)] * 4
                    else:
                        t4 = sb - n16
                        sts = [stage_st(n16 + 4 * t4 + g) for g in range(4)]
                    mv = mvp.tile([P, 16, P], BF, tag="mv")
                    if NDMA > 0:
                        nc.gpsimd.dma_gather(
                            out_ap=mv[:, :NDMA, :],
                            in_ap=btd_d[:],
                            idxs_ap=bidx16[:, sb * ncols16 : (sb + 1) * ncols16],
                            num_idxs=NDMA * P,
                            num_idxs_reg=vnum,
                            elem_size=P,
                        )
                    gather(nc.vector, vregs, mv, s0, NDMA, nV)
                    gather(nc.scalar, aregs, mv, s0, NDMA + nV, nA)
                    gather(nc.gpsimd, gregs, mv, s0, NDMA + nV + nA, nG)
                    pts = [psP.tile([P, 1024], FP, tag="pp", name=f"pp{h}")
                           for h in range(2)]
                    for q in range(4):
                        _mm(nc, pts[q // 2][:, (q % 2) * 512 : (q % 2 + 1) * 512],
                            sts[q][:],
                            mv[:, 4 * q : 4 * q + 4, :].rearrange("r m c -> r (m c)"))
                    ob = obufp.tile([P, 16 * P], BF, tag="ob")
                    # vector evacs ob[:, :EVAC_DVE], scalar the rest
                    lo = min(EVAC_DVE, 1024)
                    if lo > 0:
                        nc.vector.tensor_copy(ob[:, :lo], pts[0][:, :lo])
                    if lo < 1024:
                        nc.scalar.copy(ob[:, lo:1024], pts[0][:, lo:])
                    hi = max(EVAC_DVE, 1024)
                    if hi > 1024:
                        nc.vector.tensor_copy(
                            ob[:, 1024:hi], pts[1][:, : hi - 1024])
                    if hi < 2048:
                        nc.scalar.copy(ob[:, hi:], pts[1][:, hi - 1024 :])
                    nc.sync.dma_start(
                        out_d[:, s0 * P : (s0 + 16) * P], ob[:]
                    )
    nc.compile()
    return nc


def _plan_blocks(unique):
    """Pack positions into 16-blocks and 4-blocks sharing a stationary entry."""
    lo = unique & 255
    hi = unique >> 8
    ent = np.where(hi > 0, lo, IDENT_ENTRY)
    bent = np.where(hi > 0, hi, unique)  # hi==0 -> out = I @ M(pos)
    order = np.argsort(ent, kind="stable")
    es = ent[order]
    bounds = np.flatnonzero(np.r_[True, es[1:] != es[:-1], True])

    blocks16, blocks4 = [], []
    for s, e in zip(bounds[:-1], bounds[1:]):
        idxs = order[s:e]
        v = int(es[s])
        g = len(idxs)
        q0 = 0
        while g - q0 >= 16:
            blocks16.append((v, idxs[q0 : q0 + 16]))
            q0 += 16
        while q0 < g:
            blocks4.append((v, idxs[q0 : q0 + 4]))
            q0 += 4
    return blocks16, blocks4, bent


def kernel(unique, primitives_raw, identity=None, **_):
    unique_full = np.asarray(unique)
    praw = np.ascontiguousarray(np.asarray(primitives_raw, np.float32))

    # duplicate positions compute identical outputs: plan on the unique set
    # and fan the result back out through slot_of_pos
    uniq, inv = np.unique(unique_full.astype(np.int64), return_inverse=True)
    unique = uniq

    blocks16, blocks4, bent = _plan_blocks(unique)
    n16 = -(-len(blocks16) // NCORES)
    n4 = -(-len(blocks4) // (NCORES * 4)) * 4
    while len(blocks16) < NCORES * n16:
        blocks16.append((IDENT_ENTRY, np.empty(0, np.int64)))
    while len(blocks4) < NCORES * n4:
        blocks4.append((IDENT_ENTRY, np.empty(0, np.int64)))
    nslots = n16 * 16 + n4 * 4

    nsb = n16 + n4 // 4
    ncols16 = NDMA * P // 16
    slot_of_pos = np.zeros(unique.shape[0], np.int64)
    sioff = np.zeros((NCORES, n16 + n4), np.int32)
    ent_of_slot = np.zeros((NCORES, nslots), np.int32)  # bent per slot
    for i, (v, mem) in enumerate(blocks16):
        c, k = divmod(i, n16)
        sioff[c, k] = v * P
        for j, pidx in enumerate(mem):
            ent_of_slot[c, k * 16 + j] = int(bent[pidx])
            slot_of_pos[pidx] = c * nslots + k * 16 + j
    for i, (v, mem) in enumerate(blocks4):
        c, k = divmod(i, n4)
        sioff[c, n16 + k] = v * P
        base = n16 * 16 + k * 4
        for j, pidx in enumerate(mem):
            ent_of_slot[c, base + j] = int(bent[pidx])
            slot_of_pos[pidx] = c * nslots + base + j
    # engine-gather offsets: row 0 in bf16 elems (*128), row 1 in int32
    # elems (*64) for the bitcast copies
    bidx = np.stack(
        [ent_of_slot * P, ent_of_slot * (P // 2)], axis=1
    ).astype(np.int32)                                           # [C, 2, ns]
    bidx16 = None
    if NDMA > 0:
        # dma_gather int16 indices: slot g of superblock sb -> 128 indices
        # i = g*128+p valued p*256+e, laid out at [i%16, sb*ncols16 + i//16]
        es = ent_of_slot.reshape(NCORES, nsb, 16)[:, :, :NDMA]
        p = np.arange(P)
        idxval = (p[None, None, None, :] * NB + es[..., None]).astype(np.int64)
        i_lin = (np.arange(NDMA)[:, None] * P + p[None, :])      # [NDMA, P]
        bidx16 = np.zeros((NCORES, 16, nsb * ncols16), np.int16)
        rows = (i_lin % 16).ravel()
        cols = (i_lin // 16).ravel()
        for c in range(NCORES):
            for sb in range(nsb):
                vals = idxval[c, sb].ravel()                     # [NDMA*P]
                bidx16[c, rows, sb * ncols16 + cols] = vals
        # each Q7 core pair reads its own partitions: replicate the block 8x
        bidx16 = np.tile(bidx16, (1, 8, 1))                      # [C, 128, ..]

    key = (n16, n4)
    if key not in _prog_cache:
        _prog_cache[key] = build_program(n16, n4)
    nc = _prog_cache[key]

    in_maps = [
        {
            "praw": praw,
            "sioff": np.ascontiguousarray(sioff[c].reshape(1, -1)),
            "bidx": np.ascontiguousarray(bidx[c]),
            **({"bidx16": np.ascontiguousarray(bidx16[c])} if NDMA > 0 else {}),
        }
        for c in range(NCORES)
    ]
    global _last_ctx
    _last_ctx = (nc, in_maps)
    res = run_bass_kernel_spmd(nc, in_maps, list(range(NCORES)))
    outs = np.concatenate(
        [
            np.asarray(res.results[c]["out"])
            .reshape(P, nslots, P)
            .transpose(1, 0, 2)
            for c in range(NCORES)
        ],
        axis=0,
    )
    return np.ascontiguousarray(outs[slot_of_pos[inv]]).astype(np.float32)


if __name__ == "__main__":
    rng = np.random.default_rng(0)
    u = rng.integers(1, 65536, 64).astype(np.int32)
    pr = rng.random((2, P, P), np.float32)
    o = kernel(u, pr)
    print(o.shape, o.dtype)


# revision 43
# speedup vs baseline: 1.1360x; 1.0070x over previous
"""Trainium2 Bass kernel for nn_BinaryPathEncoder.

Math: out[n] = prod_k W_{b_k(pos_n)}^T (product over the binary digits of
pos_n below its leading 1; W_0/W_1 = expm(herm_b), pad -> identity).

Let G_b = W_b^T = expm(-herm_b), M(h) = G_{b_0(h)} @ G_{b_1(h)} @ ...
Split pos = hi*256 + lo:
  hi >= 1:  out = A8(lo) @ M(hi)   (8 low bits all valid)
  hi == 0:  out = I @ M(pos)
Tables (per core, identical SPMD program):
  - G via scaling-squaring Taylor in fp32; G/GT stored fp32r
  - A2/A4/A4T doubling tables, M(1..15) chains: fp32r matmuls
  - stat[lo] = A8(lo)^T = A4T[lo>>4] @ A4T[lo&15] -> bf16 DRAM table
  - btab[h] = M(h): M(16q+m) = A4(m) @ M(q) -> bf16 SBUF [P, 16(q), 16(m), P]
Position loop, superblocks of 16 slots (1 block16 or 4 block4s):
  - stationary staged by dyn-offset DMA from the DRAM stat table (sync)
  - moving operands gathered from SBUF btab by dyn-offset engine copies
    split across vector/scalar/gpsimd (reg_load of index batches per engine)
  - 4 static matmuls [128,512] bf16 -> 2 PSUM [128,1024] tiles
  - evac fp32->bf16 split vector/scalar; bf16 out DMA (gpsimd issue)
Host converts bf16->fp32 and scatters slots back to input order.
"""

import contextlib
import os

import numpy as np

import concourse.bass as bass
import concourse.bacc as bacc
import concourse.mybir as mybir
import concourse.tile as tile
import concourse.tile_utils as tile_utils
tile_utils.max_sbuf_usage = 206 * 1024
from concourse.bass_utils import run_bass_kernel_spmd
from concourse.masks import make_identity

FP = mybir.dt.float32
FR = mybir.dt.float32r
BF = mybir.dt.bfloat16
I32 = mybir.dt.int32
P = 128
NCORES = 8
S_EXP = 5          # scaling-squaring: X = -H / 2^S_EXP
ORDER = 12         # Taylor order (||H||~37 -> tail ~1e-8)
NB = 256           # table entries
IDENT_ENTRY = 256  # stationary-table entry holding the identity

# slots gathered by one hardware dma_gather per superblock (from DRAM btab);
# 0 = disabled (the InstDMAGatherAnt path crashes under 8-core SPMD here)
NDMA = int(os.environ.get("NDMA", "0"))
# remaining slots gathered by engine copies (vector, scalar, gpsimd)
GSPLIT = tuple(int(x) for x in os.environ.get("GSPLIT", "6,4,6").split(","))
assert NDMA + sum(GSPLIT) == 16
# evac split: first EVAC_DVE columns (of 2048) on vector, rest on scalar
EVAC_DVE = int(os.environ.get("EVAC_DVE", "1024"))

_prog_cache = {}
_last_ctx = None


def _mm(nc, out, lhsT, rhs):
    nc.tensor.matmul(out, lhsT=lhsT, rhs=rhs, start=True, stop=True)


def _build_expm(nc, consts, psB, scratch, praw, ident):
    """Return (G, GT) fp32r tile pairs: G_b = expm(-H_b), GT_b = G_b^T.

    Interleaves the b=0/b=1 chains to hide serial latency. Taylor recurrence
    T <- I + (X/k) @ T with pre-scaled copies of X^T, identity added on DVE.
    """
    xtj, t, u = {}, {}, {}
    for b in range(2):
        pb = praw[:, b, :]
        ps_t = psB.tile([P, 512], FP, tag="psb")
        nc.tensor.transpose(out=ps_t[:, :P], in_=pb, identity=ident[:])
        xt = scratch.tile([P, P], FP, tag=f"xt{b}")
        nc.vector.tensor_tensor(
            out=xt[:], in0=pb, in1=ps_t[:, :P], op=mybir.AluOpType.subtract
        )
        xj = consts.tile([P, ORDER, P], FP, tag=f"xtj{b}")
        for k in range(1, ORDER + 1):
            s = 1.0 / ((1 << S_EXP) * k)
            if k % 2 == 0:
                nc.vector.tensor_scalar_mul(xj[:, k - 1, :], xt[:], s)
            else:
                nc.scalar.mul(xj[:, k - 1, :], xt[:], s)
        xtj[b] = xj
        tc0 = scratch.tile([P, P], FP, tag=f"tay{b}")
        nc.vector.tensor_copy(tc0[:], ident[:])
        t[b] = tc0
    for k in range(ORDER, 0, -1):
        for b in range(2):
            ps = psB.tile([P, 512], FP, tag="psb")
            _mm(nc, ps[:, :P], xtj[b][:, k - 1, :], t[b][:])
            tn = scratch.tile([P, P], FP, tag=f"tay{b}")
            nc.vector.tensor_tensor(
                out=tn[:], in0=ps[:, :P], in1=ident[:], op=mybir.AluOpType.add
            )
            t[b] = tn
    for b in range(2):
        ps_u = psB.tile([P, 512], FP, tag="psb")
        nc.tensor.transpose(out=ps_u[:, :P], in_=t[b][:], identity=ident[:])
        uc = scratch.tile([P, P], FP, tag=f"tayu{b}")
        nc.scalar.copy(uc[:], ps_u[:, :P])
        u[b] = uc
    for _ in range(S_EXP):
        for b in range(2):
            ps1 = psB.tile([P, 512], FP, tag="psb")
            ps2 = psB.tile([P, 512], FP, tag="psb")
            _mm(nc, ps1[:, :P], u[b][:], t[b][:])   # T' = T @ T
            _mm(nc, ps2[:, :P], t[b][:], u[b][:])   # U' = (T@T)^T
            tn = scratch.tile([P, P], FP, tag=f"tay{b}")
            un = scratch.tile([P, P], FP, tag=f"tayu{b}")
            nc.vector.tensor_copy(tn[:], ps1[:, :P])
            nc.scalar.copy(un[:], ps2[:, :P])
            t[b], u[b] = tn, un
    Gs, GTs = [], []
    for b in range(2):
        g = consts.tile([P, P], FR, tag=f"g{b}")
        gt = consts.tile([P, P], FR, tag=f"gt{b}")
        nc.vector.tensor_copy(g[:], t[b][:])
        nc.scalar.copy(gt[:], u[b][:])
        Gs.append(g)
        GTs.append(gt)
    return Gs, GTs


def build_program(n16, n4):
    assert n4 % 4 == 0
    nblk = n16 + n4
    nslots = n16 * 16 + n4 * 4
    nsb = n16 + n4 // 4          # superblocks of 16 slots
    nc = bacc.Bacc("TRN2", target_bir_lowering=False, debug=False,
                   num_devices=NCORES)
    ncols16 = NDMA * P // 16     # int16 index columns per superblock
    praw_d = nc.dram_tensor("praw", [2, P, P], FP, kind="ExternalInput")
    sioff_d = nc.dram_tensor("sioff", [1, nblk], I32, kind="ExternalInput")
    # row 0: bf16-element offsets (scalar copies); row 1: int32-element
    # offsets (vector/gpsimd copies run bitcast-to-int32, halving elems)
    bidx_d = nc.dram_tensor("bidx", [2, nslots], I32, kind="ExternalInput")
    out_d = nc.dram_tensor("out", [P, nslots * P], BF, kind="ExternalOutput")
    stat_d = nc.dram_tensor("stat", [(NB + 1) * P, P], BF)
    if NDMA > 0:
        bidx16_d = nc.dram_tensor("bidx16", [P, nsb * ncols16],
                                  mybir.dt.int16, kind="ExternalInput")
        # btab DRAM copy, row (p*256 + e) = row p of entry e -> the
        # SBUF->DRAM spill is 128 contiguous 64KB runs; gather idx fits int16
        btd_d = nc.dram_tensor("btd", [P * NB, P], BF)

    with tile.TileContext(nc) as tc:
        with (
            tc.tile_pool(name="consts", bufs=1) as consts,
            tc.tile_pool(name="scratch", bufs=2) as scratch,
            tc.tile_pool(name="atab", bufs=1) as atab,
            tc.tile_pool(name="btab", bufs=1) as btabp,
            tc.tile_pool(name="sstage", bufs=4) as sstagep,
            tc.tile_pool(name="stage", bufs=int(os.environ.get("STAGE_BUFS", "8"))) as stagep,
            tc.tile_pool(name="mv", bufs=int(os.environ.get("MV_BUFS", "4"))) as mvp,
            tc.tile_pool(name="obuf", bufs=int(os.environ.get("OBUF_BUFS", "4"))) as obufp,
            tc.tile_pool(name="psB", bufs=int(os.environ.get("PSB_BUFS", "4")), space="PSUM") as psB,
            tc.tile_pool(name="psP", bufs=int(os.environ.get("PSP_BUFS", "2")), space="PSUM") as psP,
        ):
            ident = consts.tile([P, P], FP, tag="ident")
            make_identity(nc, ident[:])
            praw = consts.tile([P, 2, P], FP, tag="praw")
            nc.sync.dma_start(praw[:], praw_d[:].rearrange("p r c -> r p c"))
            sioff = consts.tile([1, nblk], I32, tag="sioff")
            bidx = consts.tile([2, nslots], I32, tag="bidx")
            nc.sync.dma_start(sioff[:], sioff_d[:])
            nc.sync.dma_start(bidx[:], bidx_d[:])
            if NDMA > 0:
                bidx16 = consts.tile([P, nsb * ncols16], mybir.dt.int16,
                                     tag="bidx16")
                nc.sync.dma_start(bidx16[:], bidx16_d[:])

            # ---- phase A: primitives ----
            G, GT = _build_expm(nc, consts, psB, scratch, praw, ident)

            # ---- phases A2+B interleaved: M(1..15) chains and A2/A4 tables
            # (fp32r).  The m15 levels are serial (mm -> evac -> mm); the a2/a4
            # mms are emitted between the levels so the PE never stalls on the
            # m15 evacs (tile executes per-engine in emission order).
            m15 = atab.tile([P, 16, P], FR, tag="m15")
            btab = btabp.tile([P, 16, 16, P], BF, tag="btab")
            nc.vector.tensor_copy(m15[:, 1, :], ident[:])
            nc.vector.tensor_copy(m15[:, 2, :], G[0][:])
            nc.vector.tensor_copy(m15[:, 3, :], G[1][:])
            nc.scalar.copy(btab[:, 0, 0, :], ident[:])
            nc.scalar.copy(btab[:, 0, 1, :], ident[:])
            nc.scalar.copy(btab[:, 0, 2, :], G[0][:])
            nc.scalar.copy(btab[:, 0, 3, :], G[1][:])
            m15v = m15[:].rearrange("r (c b2) p -> r c b2 p", b2=2)
            btsv = btab[:, 0, :, :].rearrange("r (c b2) p -> r c b2 p", b2=2)

            def m15_level(c0, c1):
                # entries 2c+b for c in [c0, c1): M(2c+b) = G_b @ M(c)
                ncols = c1 - c0
                for b in range(2):
                    ps = psB.tile([P, 512], FP, tag="psb")
                    _mm(nc, ps[:, : ncols * P], GT[b][:],
                        m15[:, c0:c1, :].rearrange("r m c -> r (m c)"))
                    psv = ps[:, : ncols * P].rearrange("r (m c) -> r m c", c=P)
                    nc.vector.tensor_copy(m15v[:, c0:c1, b, :], psv)
                    nc.scalar.copy(btsv[:, c0:c1, b, :], psv)

            a2 = atab.tile([P, 4, P], FR, tag="a2")
            a2t = atab.tile([P, 4, P], FR, tag="a2t")
            m15_level(2, 4)
            for m in range(4):
                ps = psB.tile([P, 512], FP, tag="psb")
                _mm(nc, ps[:, :P], GT[m & 1][:], G[m >> 1][:])   # A2[m]
                nc.vector.tensor_copy(a2[:, m, :], ps[:, :P])
                ps2 = psB.tile([P, 512], FP, tag="psb")
                _mm(nc, ps2[:, :P], G[m >> 1][:], GT[m & 1][:])  # A2T[m]
                nc.scalar.copy(a2t[:, m, :], ps2[:, :P])
                if m == 1:
                    m15_level(4, 8)
            a4 = atab.tile([P, 16, P], FR, tag="a4")
            a4t = atab.tile([P, 16, P], FR, tag="a4t")
            a2f = a2[:].rearrange("r m c -> r (m c)")
            a2tf = a2t[:].rearrange("r m c -> r (m c)")
            for a in range(4):
                ps = psB.tile([P, 512], FP, tag="psb")
                _mm(nc, ps[:], a2t[:, a, :], a2f)        # A4[a+4b] over b
                for b2 in range(4):
                    nc.vector.tensor_copy(
                        a4[:, a + 4 * b2, :], ps[:, b2 * P : (b2 + 1) * P]
                    )
                # A4T[m] = A2T[m>>2] @ A2T[m&3]; fix a=m>>2: m = 4a+b contiguous
                ps2 = psB.tile([P, 512], FP, tag="psb")
                _mm(nc, ps2[:], a2[:, a, :], a2tf)
                nc.scalar.copy(
                    a4t[:, 4 * a : 4 * a + 4, :].rearrange("r m c -> r (m c)"),
                    ps2[:],
                )

            # ---- phase C: stat table (A8^T) -> stat_d rows (bf16) ----
            a4tf = a4t[:].rearrange("r m c -> r (m c)")
            stat_v = stat_d[:].rearrange("(e r) c -> r e c", r=P)
            for g in range(16):
                for q in range(4):
                    sst = sstagep.tile([P, 4, P], BF, tag="sst")
                    ps = psB.tile([P, 512], FP, tag="psb")
                    # stat[16g + (4q+j)] = A4T[g] @ A4T[4q+j], j=0..3
                    _mm(nc, ps[:], a4[:, g, :], a4tf[:, q * 512 : (q + 1) * 512])
                    psv = ps[:].rearrange("r (m c) -> r m c", c=P)
                    if (g + q) % 2 == 0:
                        nc.vector.tensor_copy(sst[:], psv)
                    else:
                        nc.scalar.copy(sst[:], psv)
                    nc.sync.dma_start(
                        stat_v[:, 16 * g + 4 * q : 16 * g + 4 * q + 4, :], sst[:]
                    )
            sstI = sstagep.tile([P, 4, P], BF, tag="sst")
            nc.vector.tensor_copy(sstI[:, 0, :], ident[:])
            nc.sync.dma_start(stat_v[:, NB : NB + 1, :], sstI[:, 0:1, :])

            # ---- phase D: btab entries 16..255 = A4(m) @ M(q), fp32r ----
            for m in range(16):
                for (q0, q1) in ((1, 5), (5, 9), (9, 13), (13, 16)):
                    nq = q1 - q0
                    ps = psB.tile([P, 512], FP, tag="psb")
                    _mm(nc, ps[:, : nq * P], a4t[:, m, :],
                        m15[:, q0:q1, :].rearrange("r m c -> r (m c)"))
                    psv = ps[:, : nq * P].rearrange("r (m c) -> r m c", c=P)
                    if (m + q0) % 2 == 0:
                        nc.vector.tensor_copy(btab[:, q0:q1, m, :], psv)
                    else:
                        nc.scalar.copy(btab[:, q0:q1, m, :], psv)

            if NDMA > 0:
                # spill btab to DRAM rows (p*256+e): 64KB runs per partition
                nc.sync.dma_start(
                    btd_d[:].rearrange("(r e) c -> r (e c)", r=P), btab[:]
                )

            # ---- phase E: position loop ----
            btf = btab[:].rearrange("r q m p -> r (q m p)")
            nV, nA, nG = GSPLIT
            with contextlib.ExitStack() as regctx:
                vregs = [regctx.enter_context(nc.vector.register(f"rv{j}"))
                         for j in range(2 * nV)]
                aregs = [regctx.enter_context(nc.scalar.register(f"ra{j}"))
                         for j in range(2 * nA)]
                gregs = [regctx.enter_context(nc.gpsimd.register(f"rg{j}"))
                         for j in range(2 * nG)]
                rs = regctx.enter_context(nc.sync.register("rs"))
                if NDMA > 0:
                    rnum = regctx.enter_context(nc.gpsimd.register("rnum"))
                    nc.gpsimd.reg_mov(rnum, NDMA * P)
                    vnum = nc.gpsimd.snap(rnum, donate=True,
                                          min_val=NDMA * P, max_val=NDMA * P)

                def stage_st(blk):
                    st = stagep.tile([P, P], BF, tag="st")
                    nc.sync.reg_load(rs, sioff[0:1, blk : blk + 1])
                    so = nc.sync.snap(rs, min_val=0, max_val=NB * P)
                    nc.sync.dma_start(st[:], stat_d[bass.ds(so, P), :])
                    return st

                btf32 = btf.bitcast(I32)        # [P, 16384] int32 view
                bidx_sb = [
                    bidx[r : r + 1, :].rearrange("a (b s) -> a b s", s=16)
                    for r in range(2)
                ]

                def load_pair(eng, regs, sb, j0, cnt):
                    # one reg_load fetches this engine's indices for sbs
                    # {sb, sb+1}; regs[cnt:] hold the odd sb's values
                    if cnt == 0:
                        return
                    row = 0 if eng is nc.scalar else 1
                    nspan = 2 if sb + 1 < nsb else 1
                    eng.reg_load(
                        regs[: nspan * cnt],
                        bidx_sb[row][:, sb : sb + nspan, j0 : j0 + cnt],
                    )

                def gather(eng, regs, mv, half, j0, cnt):
                    for i in range(cnt):
                        r = regs[half * cnt + i]
                        if eng is nc.scalar:
                            off = eng.snap(r, donate=True,
                                           min_val=0, max_val=(NB - 1) * P)
                            eng.copy(mv[:, j0 + i, :], btf[:, bass.ds(off, P)])
                        else:
                            off = eng.snap(r, donate=True,
                                           min_val=0, max_val=(NB - 1) * P // 2)
                            eng.tensor_copy(
                                mv[:, j0 + i, :].bitcast(I32),
                                btf32[:, bass.ds(off, P // 2)],
                            )

                for sb in range(nsb):
                    s0 = sb * 16
                    if sb % 2 == 0:
                        load_pair(nc.vector, vregs, sb, NDMA, nV)
                        load_pair(nc.scalar, aregs, sb, NDMA + nV, nA)
                        load_pair(nc.gpsimd, gregs, sb, NDMA + nV + nA, nG)
                    half = sb % 2
                    if sb < n16:
                        sts = [stage_st(sb)] * 4
                    else:
                        t4 = sb - n16
                        sts = [stage_st(n16 + 4 * t4 + g) for g in range(4)]
                    mv = mvp.tile([P, 16, P], BF, tag="mv")
                    if NDMA > 0:
                        nc.gpsimd.dma_gather(
                            out_ap=mv[:, :NDMA, :],
                            in_ap=btd_d[:],
                            idxs_ap=bidx16[:, sb * ncols16 : (sb + 1) * ncols16],
                            num_idxs=NDMA * P,
                            num_idxs_reg=vnum,
                            elem_size=P,
                        )
                    gather(nc.vector, vregs, mv, half, NDMA, nV)
                    gather(nc.scalar, aregs, mv, half, NDMA + nV, nA)
                    gather(nc.gpsimd, gregs, mv, half, NDMA + nV + nA, nG)
                    pts = [psP.tile([P, 1024], FP, tag="pp", name=f"pp{h}")
                           for h in range(2)]
                    for q in range(4):
                        _mm(nc, pts[q // 2][:, (q % 2) * 512 : (q % 2 + 1) * 512],
                            sts[q][:],
                            mv[:, 4 * q : 4 * q + 4, :].rearrange("r m c -> r (m c)"))
                    ob = obufp.tile([P, 16 * P], BF, tag="ob")
                    # vector evacs ob[:, :EVAC_DVE], scalar the rest
                    lo = min(EVAC_DVE, 1024)
                    if lo > 0:
                        nc.vector.tensor_copy(ob[:, :lo], pts[0][:, :lo])
                    if lo < 1024:
                        nc.scalar.copy(ob[:, lo:1024], pts[0][:, lo:])
                    hi = max(EVAC_DVE, 1024)
                    if hi > 1024:
                        nc.vector.tensor_copy(
                            ob[:, 1024:hi], pts[1][:, : hi - 1024])
                    if hi < 2048:
                        nc.scalar.copy(ob[:, hi:], pts[1][:, hi - 1024 :])
                    nc.sync.dma_start(
                        out_d[:, s0 * P : (s0 + 16) * P], ob[:]
                    )
    nc.compile()
    return nc


def _plan_blocks(unique):
    """Pack positions into 16-blocks and 4-blocks sharing a stationary entry."""
    lo = unique & 255
    hi = unique >> 8
    ent = np.where(hi > 0, lo, IDENT_ENTRY)
    bent = np.where(hi > 0, hi, unique)  # hi==0 -> out = I @ M(pos)
    order = np.argsort(ent, kind="stable")
    es = ent[order]
    bounds = np.flatnonzero(np.r_[True, es[1:] != es[:-1], True])

    blocks16, blocks4 = [], []
    for s, e in zip(bounds[:-1], bounds[1:]):
        idxs = order[s:e]
        v = int(es[s])
        g = len(idxs)
        q0 = 0
        while g - q0 >= 16:
            blocks16.append((v, idxs[q0 : q0 + 16]))
            q0 += 16
        while q0 < g:
            blocks4.append((v, idxs[q0 : q0 + 4]))
            q0 += 4
    return blocks16, blocks4, bent


def kernel(unique, primitives_raw, identity=None, **_):
    unique_full = np.asarray(unique)
    praw = np.ascontiguousarray(np.asarray(primitives_raw, np.float32))

    # duplicate positions compute identical outputs: plan on the unique set
    # and fan the result back out through slot_of_pos
    uniq, inv = np.unique(unique_full.astype(np.int64), return_inverse=True)
    unique = uniq

    blocks16, blocks4, bent = _plan_blocks(unique)
    n16 = -(-len(blocks16) // NCORES)
    n4 = -(-len(blocks4) // (NCORES * 4)) * 4
    while len(blocks16) < NCORES * n16:
        blocks16.append((IDENT_ENTRY, np.empty(0, np.int64)))
    while len(blocks4) < NCORES * n4:
        blocks4.append((IDENT_ENTRY, np.empty(0, np.int64)))
    nslots = n16 * 16 + n4 * 4

    nsb = n16 + n4 // 4
    ncols16 = NDMA * P // 16
    slot_of_pos = np.zeros(unique.shape[0], np.int64)
    sioff = np.zeros((NCORES, n16 + n4), np.int32)
    ent_of_slot = np.zeros((NCORES, nslots), np.int32)  # bent per slot
    for i, (v, mem) in enumerate(blocks16):
        c, k = divmod(i, n16)
        sioff[c, k] = v * P
        for j, pidx in enumerate(mem):
            ent_of_slot[c, k * 16 + j] = int(bent[pidx])
            slot_of_pos[pidx] = c * nslots + k * 16 + j
    for i, (v, mem) in enumerate(blocks4):
        c, k = divmod(i, n4)
        sioff[c, n16 + k] = v * P
        base = n16 * 16 + k * 4
        for j, pidx in enumerate(mem):
            ent_of_slot[c, base + j] = int(bent[pidx])
            slot_of_pos[pidx] = c * nslots + base + j
    # engine-gather offsets: row 0 in bf16 elems (*128), row 1 in int32
    # elems (*64) for the bitcast copies
    bidx = np.stack(
        [ent_of_slot * P, ent_of_slot * (P // 2)], axis=1
    ).astype(np.int32)                                           # [C, 2, ns]
    bidx16 = None
    if NDMA > 0:
        # dma_gather int16 indices: slot g of superblock sb -> 128 indices
        # i = g*128+p valued p*256+e, laid out at [i%16, sb*ncols16 + i//16]
        es = ent_of_slot.reshape(NCORES, nsb, 16)[:, :, :NDMA]
        p = np.arange(P)
        idxval = (p[None, None, None, :] * NB + es[..., None]).astype(np.int64)
        i_lin = (np.arange(NDMA)[:, None] * P + p[None, :])      # [NDMA, P]
        bidx16 = np.zeros((NCORES, 16, nsb * ncols16), np.int16)
        rows = (i_lin % 16).ravel()
        cols = (i_lin // 16).ravel()
        for c in range(NCORES):
            for sb in range(nsb):
                vals = idxval[c, sb].ravel()                     # [NDMA*P]
                bidx16[c, rows, sb * ncols16 + cols] = vals
        # each Q7 core pair reads its own partitions: replicate the block 8x
        bidx16 = np.tile(bidx16, (1, 8, 1))                      # [C, 128, ..]

    key = (n16, n4)
    if key not in _prog_cache:
        _prog_cache[key] = build_program(n16, n4)
    nc = _prog_cache[key]

    in_maps = [
        {
            "praw": praw,
            "sioff": np.ascontiguousarray(sioff[c].reshape(1, -1)),
            "bidx": np.ascontiguousarray(bidx[c]),
            **({"bidx16": np.ascontiguousarray(bidx16[c])} if NDMA > 0 else {}),
        }
        for c in range(NCORES)
    ]
    global _last_ctx
    _last_ctx = (nc, in_maps)
    res = run_bass_kernel_spmd(nc, in_maps, list(range(NCORES)))
    outs = np.concatenate(
        [
            np.asarray(res.results[c]["out"])
            .reshape(P, nslots, P)
            .transpose(1, 0, 2)
            for c in range(NCORES)
        ],
        axis=0,
    )
    return np.ascontiguousarray(outs[slot_of_pos[inv]]).astype(np.float32)


if __name__ == "__main__":
    rng = np.random.default_rng(0)
    u = rng.integers(1, 65536, 64).astype(np.int32)
    pr = rng.random((2, P, P), np.float32)
    o = kernel(u, pr)
    print(o.shape, o.dtype)
